# revision 38
# baseline (speedup 1.0000x reference)
"""Causal self-attention (B=2, T=2048, D=2048, 16 heads) on 8 trn2 cores.

Sharding: tensor-parallel over heads - 2 heads per core. Each core computes
q/k/v projections for its 2 heads (column-parallel), causal attention per
head, and a partial output projection (row-parallel). Host sums the 8
partial outputs.

Design notes (v3, single interleaved stream):
  - projections are decomposed into 2-matmul micro-units kept in a FIFO;
    attention chunks pull one unit per j-tile so the PE never stalls on
    the scalar exp chain, and drain the FIFO between chunks to satisfy
    the causal data dependencies (attn group ic needs proj chunks <= ic).
  - attention for batch 0 starts right after proj chunk 0 (~15us) instead
    of after all projections; batch 1 runs ic descending so the tail ends
    on the smallest group.
  - v is produced directly in [token, d] layout (x-tile stationary) - no
    PE transposes.
  - out-projection units fill attention bubbles once the proj FIFO is
    empty; psum casts rotate vector/scalar/gpsimd so the drain streams.
  - 8 psum banks: pa,pb (proj q/k + outproj), va,vb (v + outproj drain),
    s0,s1 (scores + denominator rows), o0,o1 (PV accumulate).
"""

import math
from contextlib import ExitStack

import numpy as np
import ml_dtypes

import concourse.bass as bass
import concourse.bass_isa as bass_isa
import concourse.mybir as mybir
import concourse.tile as tile
from concourse import bacc
from concourse.bass_utils import run_bass_kernel_spmd

P = 128
D_MODEL = 2048
NUM_HEADS = 16
D = 128            # head dim
B, T = 2, 2048
BT = B * T         # 4096
NCORES = 8
HPC = NUM_HEADS // NCORES   # 2 heads per core
KD = D_MODEL // P           # 16 d_model tiles
TJ = T // P                 # 16 key tiles per batch
IC = 512                    # query / token chunk width
NI = T // IC                # 4 query chunks per batch
TCH = BT // IC              # 8 token chunks for projections

F32 = mybir.dt.float32
BF16 = mybir.dt.bfloat16
NWARM = 8                   # PE p-state warmup matmuls

CFG_BF16 = dict()
CFG_SAFE = CFG_FAST = CFG_F32R = CFG_BF16


def _emit(tc, xT, wqT, wkT, wvT, woT, y):
    nc = tc.nc
    scale = 1.0 / math.sqrt(D)

    with ExitStack() as ctx:
        consts = ctx.enter_context(tc.tile_pool(name="consts", bufs=1))
        wpool = ctx.enter_context(tc.tile_pool(name="wpool", bufs=1))
        xpool = ctx.enter_context(tc.tile_pool(name="xpool", bufs=2))
        arrs = ctx.enter_context(tc.tile_pool(name="arrs", bufs=1))
        ptpool = ctx.enter_context(tc.tile_pool(name="ptpool", bufs=2))
        smalls = ctx.enter_context(tc.tile_pool(name="smalls", bufs=2))
        ypool = ctx.enter_context(tc.tile_pool(name="ypool", bufs=5))
        psum = ctx.enter_context(tc.tile_pool(name="psum", bufs=1, space="PSUM"))

        # ---- constants (wtmp first: warmup matmuls gate on it) ----
        wtmp = consts.tile([P, IC], BF16, tag="wtmp", name="wtmp")
        nc.vector.memset(wtmp, 0.125)
        ones_col = consts.tile([P, 1], BF16, tag="ones", name="ones")
        nc.vector.memset(ones_col, 1.0)
        # tri_mask[p, i] = 1.0 if i >= p else 0 (keep lower triangle of S)
        tri_mask = consts.tile([P, P], BF16, tag="trimask", name="trimask")
        nc.gpsimd.memset(tri_mask, 0.0)
        nc.gpsimd.affine_select(
            out=tri_mask, in_=tri_mask, compare_op=mybir.AluOpType.is_gt,
            fill=1.0, base=0, pattern=[[-1, P]], channel_multiplier=1,
        )

        # ---- weight + x DMAs (host pre-tiled: contiguous per partition) ----
        w_sb = {}
        for nm in ("q", "k", "v"):
            w_sb[nm] = wpool.tile([P, KD, HPC * D], BF16, tag=f"w{nm}",
                                  name=f"w{nm}")
        xt_tiles = {}

        def issue_x_dma(c, parts=1):
            xt = xpool.tile([P, KD, IC], BF16, tag="xt", name=f"xt{c}")
            src = xT[c * P:(c + 1) * P].rearrange("p (ko t) -> p ko t", ko=KD)
            kq = KD // parts
            for pp in range(parts):
                nc.sync.dma_start(xt[:, pp * kq:(pp + 1) * kq],
                                  src[:, pp * kq:(pp + 1) * kq])
            xt_tiles[c] = xt

        # first chunk's inputs stream in interleaved parts so the first
        # projection matmuls start as soon as part 0 lands and chunk 0's
        # kt-interleaved consumption never outruns the DMA stream
        wq_src = wqT.rearrange("p (ko o) -> p ko o", ko=KD)
        xt0 = xpool.tile([P, KD, IC], BF16, tag="xt", name="xt0")
        x0_src = xT[0:P].rearrange("p (ko t) -> p ko t", ko=KD)
        nc.sync.dma_start(w_sb["q"][:, 0:4], wq_src[:, 0:4])
        nc.sync.dma_start(xt0[:, 0:4], x0_src[:, 0:4])
        nc.sync.dma_start(w_sb["q"][:, 4:8], wq_src[:, 4:8])
        nc.sync.dma_start(xt0[:, 4:8], x0_src[:, 4:8])
        nc.sync.dma_start(w_sb["q"][:, 8:], wq_src[:, 8:])
        nc.sync.dma_start(xt0[:, 8:12], x0_src[:, 8:12])
        nc.sync.dma_start(xt0[:, 12:], x0_src[:, 12:])
        xt_tiles[0] = xt0
        nc.sync.dma_start(w_sb["k"], wkT.rearrange("p (ko o) -> p ko o", ko=KD))
        nc.sync.dma_start(w_sb["v"], wvT.rearrange("p (ko o) -> p ko o", ko=KD))
        wo_sb = wpool.tile([P, HPC, D_MODEL], BF16, tag="wo", name="wo")
        nc.sync.dma_start(wo_sb, woT.rearrange("p (h m) -> p h m", h=HPC))

        # ---- PE warmup: ramp p-state while the first DMAs stream ----
        ps_w = psum.tile([P, IC], F32, tag="pa", name="warm")
        for _ in range(NWARM):
            nc.tensor.matmul(ps_w, wtmp[:, :P], wtmp, start=True, stop=True)

        # ---- persistent sbuf arrays ----
        qT = [arrs.tile([P, BT], BF16, tag=f"qT{h}", name=f"qT{h}")
              for h in range(HPC)]
        kT = [arrs.tile([P, BT], BF16, tag=f"kT{h}", name=f"kT{h}")
              for h in range(HPC)]
        v2_sb = arrs.tile([P, B, TJ, HPC, D], BF16, tag="v2", name="v2")
        outT = [arrs.tile([P, BT], BF16, tag=f"o{h}", name=f"o{h}")
                for h in range(HPC)]

        # ================= projection micro-unit FIFO =================
        # each unit is a closure emitting ~2 matmuls; markers record chunk
        # completion so attention can wait for its causal prerequisites.
        fifo = []
        done = set()

        def build_chunk(c, cnext):
            items = []
            b = c // 4
            jt0 = (c % 4) * 4
            tsl = slice(c * IC, (c + 1) * IC)

            if cnext is not None:
                items.append(lambda c=cnext: issue_x_dma(c))

            # q/k dests: [128 d, 512 tokens] accumulated over 16 k-tiles;
            # q dests first so chunk 0 matches weight DMA arrival order
            for di, (nm, h) in enumerate(
                    (("q", 0), ("q", 1), ("k", 0), ("k", 1))):
                tag = "pa" if di % 2 == 0 else "pb"
                st = {}
                for u in range(KD // 2):
                    def f(u=u, nm=nm, h=h, tag=tag, st=st, c=c):
                        if u == 0:
                            st["ps"] = psum.tile([P, IC], F32, tag=tag,
                                                 name=f"pj{c}")
                        ps = st["ps"]
                        for kt in (2 * u, 2 * u + 1):
                            nc.tensor.matmul(
                                ps, w_sb[nm][:, kt, h * D:(h + 1) * D],
                                xt_tiles[c][:, kt],
                                start=(kt == 0), stop=(kt == KD - 1),
                                skip_group_check=True)
                    items.append(f)

                def fcopy(nm=nm, h=h, st=st, tsl=tsl):
                    dst = qT[h] if nm == "q" else kT[h]
                    nc.vector.tensor_copy(dst[:, tsl], st["ps"])
                items.append(fcopy)

            # v dests: [128 tokens, 256 d] directly transposed (x stationary)
            for t4 in range(4):
                tag = "va" if t4 % 2 == 0 else "vb"
                st = {}
                for u in range(KD // 2):
                    def f(u=u, t4=t4, tag=tag, st=st, c=c):
                        if u == 0:
                            st["ps"] = psum.tile([P, IC], F32, tag=tag,
                                                 name=f"pv{c}")
                        ps = st["ps"]
                        for kt in (2 * u, 2 * u + 1):
                            nc.tensor.matmul(
                                ps[:, :HPC * D],
                                xt_tiles[c][:, kt, t4 * P:(t4 + 1) * P],
                                w_sb["v"][:, kt],
                                start=(kt == 0), stop=(kt == KD - 1),
                                skip_group_check=True)
                    items.append(f)

                def fcopy(t4=t4, st=st, b=b, jt0=jt0):
                    for h in range(HPC):
                        nc.vector.tensor_copy(
                            v2_sb[:, b, jt0 + t4, h],
                            st["ps"][:, h * D:(h + 1) * D])
                items.append(fcopy)

            items.append(lambda c=c: done.add(c))
            return items

        # ============== out-projection units + cast queue ==============
        unit_pool = []
        cast_q = []
        ustate = {"u": 0, "c": 0, "y_sb": None, "deep": False}

        def make_units(ic, b):
            t0 = (b * T + ic * IC) // P
            for tt in range(t0, t0 + IC // P):
                for mc in range(D_MODEL // IC):
                    unit_pool.append((tt, mc))

        def emit_cast():
            # gpsimd cannot read PSUM: casts alternate vector/scalar
            tt, mc, y_sb, ps_y, u = cast_q.pop(0)
            msl = slice(mc * IC, (mc + 1) * IC)
            c = ustate["c"]; ustate["c"] += 1
            # during attention scalar is exp-paced, so vector takes 2/3 of
            # the casts; in the exp-free final drain both engines are free
            # and a 50/50 split keeps vector off the critical path
            on_vec = (c % 2 == 0) if ustate["deep"] else (c % 3 != 2)
            if on_vec:
                nc.vector.tensor_copy(y_sb[:, msl], ps_y)
            else:
                nc.scalar.copy(y_sb[:, msl], ps_y)
            # dma_start costs ~0.6us of sync-queue time regardless of size;
            # two column-half DMAs per token tile spread all 16 engines
            if mc == D_MODEL // IC - 1:
                half = D_MODEL // 2
                nc.sync.dma_start(y[tt * P:(tt + 1) * P, :half],
                                  y_sb[:, :half])
                nc.sync.dma_start(y[tt * P:(tt + 1) * P, half:],
                                  y_sb[:, half:])

        def emit_unit(deep):
            # casts are deferred one slot so the cast queues never block on
            # ps_y matmuls (convoy avoidance)
            tt, mc = unit_pool.pop(0)
            u = ustate["u"]; ustate["u"] += 1
            msl = slice(mc * IC, (mc + 1) * IC)
            if mc == 0:
                ustate["y_sb"] = ypool.tile([P, D_MODEL], BF16, tag="ysb",
                                            name="ysb")
            y_sb = ustate["y_sb"]
            # units are only emitted once the proj FIFO is empty, so all
            # four proj banks are free for a deep rotation
            ytag = ["pa", "pb", "va", "vb"][u % 4]
            ps_y = psum.tile([P, IC], F32, tag=ytag, name="yps")
            for h in range(HPC):
                nc.tensor.matmul(
                    ps_y, outT[h][:, tt * P:(tt + 1) * P], wo_sb[:, h, msl],
                    start=(h == 0), stop=(h == HPC - 1))
            cast_q.append((tt, mc, y_sb, ps_y, u))
            if deep:
                emit_cast()

        def pull():
            # one PE filler micro-unit: proj FIFO first, then outproj units
            if fifo:
                fifo.pop(0)()
            elif unit_pool:
                emit_unit(deep=False)

        def drain_until(c):
            while c not in done and fifo:
                fifo.pop(0)()

        # ===================== attention chunk =====================
        def attn_chunk(ck, ic, h, b):
            i0 = b * T + ic * IC
            nj = 4 * (ic + 1)
            lo_of = lambda jt: max(jt - 4 * ic, 0) * P
            pt = ptpool.tile([P, 16 * IC], BF16, tag="pt", name="pt")

            def s_tile(jt):
                lo = lo_of(jt)
                ps_s = psum.tile([P, IC], F32, tag=f"s{jt % 2}",
                                 name=f"s{jt % 2}")
                nc.tensor.matmul(
                    ps_s[:, lo:],
                    kT[h][:, b * T + jt * P: b * T + (jt + 1) * P],
                    qT[h][:, i0 + lo: i0 + IC], start=True, stop=True)
                nc.scalar.activation(
                    pt[:, jt * IC + lo:(jt + 1) * IC], ps_s[:, lo:],
                    mybir.ActivationFunctionType.Exp, scale=scale)
                if jt - 4 * ic >= 0:
                    nc.vector.tensor_tensor(
                        pt[:, jt * IC + lo: jt * IC + lo + P],
                        pt[:, jt * IC + lo: jt * IC + lo + P],
                        tri_mask, mybir.AluOpType.mult)

            # denominator accumulates on DVE (bf16), freeing the PE
            pt_acc = smalls.tile([P, IC], BF16, tag="ptacc", name="ptacc")
            # PV double-buffered (o0/o1) so the norm chain of chunk n
            # overlaps chunk n+1's PV accumulation
            ps_o = psum.tile([P, IC], F32, tag=f"o{ck % 2}",
                             name=f"o{ck % 2}")

            # software pipeline: S runs one j-tile ahead of PV; a filler
            # micro-unit leads every slot so the PE queue stays dense
            s_tile(0)
            for jt in range(nj):
                if cast_q:
                    emit_cast()
                pull()
                if jt + 1 < nj:
                    s_tile(jt + 1)
                lo = lo_of(jt)
                psl = slice(jt * IC + lo, (jt + 1) * IC)
                if jt == 0:
                    nc.vector.tensor_copy(pt_acc, pt[:, psl])
                else:
                    nc.vector.tensor_tensor(
                        pt_acc[:, lo:], pt_acc[:, lo:], pt[:, psl],
                        mybir.AluOpType.add)
                nc.tensor.matmul(
                    ps_o[:, lo:], v2_sb[:, b, jt, h], pt[:, psl],
                    start=(jt == 0), stop=(jt == nj - 1),
                    skip_group_check=True)

            # denominator: one gpsimd partition all-reduce replaces the
            # ones-matmul + psum copy + broadcast chain (gpsimd is idle,
            # and this decouples the next chunk's first S matmul from the
            # denominator bank)
            bc = smalls.tile([P, IC], F32, tag="bc", name="bc")
            nc.gpsimd.partition_all_reduce(bc, pt_acc, P,
                                           bass_isa.ReduceOp.add)
            rb = smalls.tile([P, IC], F32, tag="rb", name="rb")
            nc.vector.reciprocal_approx_fast(out=rb, in_=bc)
            nc.vector.tensor_tensor(
                outT[h][:, i0:i0 + IC], ps_o, rb, mybir.AluOpType.mult)

        def emit_chunk0():
            # chunk 0 is DMA-bandwidth-bound: interleave kt across dest
            # pairs (q0/q1, then k0/k1, then v pairs) so each 0.5MB x part
            # is consumed over ~1.7us vs its ~1.4us transfer
            issue_x_dma(4)
            # all 8 psum banks are free during chunk 0, so each pair gets
            # fresh banks and no copy-vs-matmul WAR bubbles
            for pair, tags in (((("q", 0), ("q", 1)), ("pa", "pb")),
                               ((("k", 0), ("k", 1)), ("s0", "s1"))):
                ps = [psum.tile([P, IC], F32, tag=t, name="pj0") for t in tags]
                for kt in range(KD):
                    for (nm, h), p in zip(pair, ps):
                        nc.tensor.matmul(
                            p, w_sb[nm][:, kt, h * D:(h + 1) * D],
                            xt_tiles[0][:, kt],
                            start=(kt == 0), stop=(kt == KD - 1),
                            skip_group_check=True)
                for (nm, h), p in zip(pair, ps):
                    dst = qT[h] if nm == "q" else kT[h]
                    nc.vector.tensor_copy(dst[:, :IC], p)
            for t4s, vtags in (((0, 1), ("va", "vb")), ((2, 3), ("o0", "o1"))):
                ps = [psum.tile([P, IC], F32, tag=t, name="pv0")
                      for t in vtags]
                for kt in range(KD):
                    for t4, p in zip(t4s, ps):
                        nc.tensor.matmul(
                            p[:, :HPC * D],
                            xt_tiles[0][:, kt, t4 * P:(t4 + 1) * P],
                            w_sb["v"][:, kt],
                            start=(kt == 0), stop=(kt == KD - 1),
                            skip_group_check=True)
                for t4, p in zip(t4s, ps):
                    for h in range(HPC):
                        nc.vector.tensor_copy(v2_sb[:, 0, t4, h],
                                              p[:, h * D:(h + 1) * D])
            done.add(0)

        # ===================== schedule =====================
        # proj chunks stream batch-interleaved (0,4,1,5,...) so that proj
        # filler work lasts until the final attention group; attention
        # groups follow the same interleave, each gated on its causal
        # prerequisite chunk via drain_until.
        cord = (0, 4, 1, 5, 2, 6, 3, 7)
        emit_chunk0()
        for i in range(1, TCH):
            cnext = cord[i + 1] if i + 1 < TCH else None
            fifo.extend(build_chunk(cord[i], cnext))

        ck = 0
        for ic in range(NI):
            for b in range(B):
                drain_until(b * 4 + ic)
                for h in range(HPC):
                    attn_chunk(ck, ic, h, b)
                    ck += 1
                make_units(ic, b)
        while fifo:
            fifo.pop(0)()
        ustate["deep"] = True
        while cast_q:
            emit_cast()
        while unit_pool:
            emit_unit(deep=True)


def _build():
    nc = bacc.Bacc("TRN2", target_bir_lowering=False, debug=False,
                   num_devices=NCORES)
    # host pre-tiles everything so each DMA is contiguous per partition
    xT = nc.dram_tensor("xT", [TCH * P, KD * IC], BF16,
                        kind="ExternalInput").ap()
    wqT = nc.dram_tensor("wqT", [P, KD * HPC * D], BF16,
                         kind="ExternalInput").ap()
    wkT = nc.dram_tensor("wkT", [P, KD * HPC * D], BF16,
                         kind="ExternalInput").ap()
    wvT = nc.dram_tensor("wvT", [P, KD * HPC * D], BF16,
                         kind="ExternalInput").ap()
    woT = nc.dram_tensor("woT", [P, HPC * D_MODEL], BF16,
                         kind="ExternalInput").ap()
    y = nc.dram_tensor("y", [BT, D_MODEL], BF16, kind="ExternalOutput").ap()
    with tile.TileContext(nc) as tc:
        _emit(tc, xT, wqT, wkT, wvT, woT, y)
    nc.compile()
    return nc


def _prep_inputs(x, Wq, Wk, Wv, Wo):
    bf = ml_dtypes.bfloat16

    def wtile(w):  # [D_MODEL, HPC*D] -> [P, KD*(HPC*D)], contiguous rows
        return np.ascontiguousarray(
            w.reshape(KD, P, HPC * D).transpose(1, 0, 2).reshape(P, -1)
        ).astype(bf)

    xT2 = np.asarray(x, np.float32).reshape(BT, D_MODEL).T  # [D_MODEL, BT]
    xT = np.ascontiguousarray(
        xT2.reshape(KD, P, TCH, IC).transpose(2, 1, 0, 3).reshape(TCH * P, -1)
    ).astype(bf)
    in_maps = []
    for c in range(NCORES):
        rows = slice(c * HPC * D, (c + 1) * HPC * D)
        woT2 = np.asarray(Wo)[:, rows].T  # [HPC*D, D_MODEL]
        in_maps.append({
            "xT": xT,
            "wqT": wtile(np.asarray(Wq)[rows].T),
            "wkT": wtile(np.asarray(Wk)[rows].T),
            "wvT": wtile(np.asarray(Wv)[rows].T),
            "woT": np.ascontiguousarray(
                woT2.reshape(HPC, P, D_MODEL).transpose(1, 0, 2)
                .reshape(P, -1)).astype(bf),
        })
    return in_maps


def run(x, Wq, Wk, Wv, Wo, cfg=None, trace=False):
    nc = _build()
    in_maps = _prep_inputs(x, Wq, Wk, Wv, Wo)
    try:
        res = run_bass_kernel_spmd(nc, in_maps, core_ids=list(range(NCORES)),
                                   trace=trace)
    except Exception:
        res = run_bass_kernel_spmd(nc, in_maps, core_ids=list(range(NCORES)),
                                   trace=trace)
    y = np.zeros((BT, D_MODEL), np.float32)
    for r in res.results:
        y += np.asarray(r["y"], dtype=np.float32)
    return y.reshape(B, T, D_MODEL), res


def kernel(x, Wq, Wk, Wv, Wo):
    y, _ = run(x, Wq, Wk, Wv, Wo)
    return y


# revision 40
# speedup vs baseline: 1.0753x; 1.0753x over previous
"""Causal self-attention (B=2, T=2048, D=2048, 16 heads) on 8 trn2 cores.

Sharding: tensor-parallel over heads - 2 heads per core. Each core computes
q/k/v projections for its 2 heads (column-parallel), causal attention per
head, and a partial output projection (row-parallel). Host sums the 8
partial outputs.

Design notes (v3, single interleaved stream):
  - projections are decomposed into 2-matmul micro-units kept in a FIFO;
    attention chunks pull one unit per j-tile so the PE never stalls on
    the scalar exp chain, and drain the FIFO between chunks to satisfy
    the causal data dependencies (attn group ic needs proj chunks <= ic).
  - attention for batch 0 starts right after proj chunk 0 (~15us) instead
    of after all projections; batch 1 runs ic descending so the tail ends
    on the smallest group.
  - v is produced directly in [token, d] layout (x-tile stationary) - no
    PE transposes.
  - out-projection units fill attention bubbles once the proj FIFO is
    empty; psum casts rotate vector/scalar/gpsimd so the drain streams.
  - 8 psum banks: pa,pb (proj q/k + outproj), va,vb (v + outproj drain),
    s0,s1 (scores + denominator rows), o0,o1 (PV accumulate).
"""

import math
from contextlib import ExitStack

import numpy as np
import ml_dtypes

import concourse.bass as bass
import concourse.bass_isa as bass_isa
import concourse.mybir as mybir
import concourse.tile as tile
from concourse import bacc
from concourse.bass_utils import run_bass_kernel_spmd

P = 128
D_MODEL = 2048
NUM_HEADS = 16
D = 128            # head dim
B, T = 2, 2048
BT = B * T         # 4096
NCORES = 8
HPC = NUM_HEADS // NCORES   # 2 heads per core
KD = D_MODEL // P           # 16 d_model tiles
TJ = T // P                 # 16 key tiles per batch
IC = 512                    # query / token chunk width
NI = T // IC                # 4 query chunks per batch
TCH = BT // IC              # 8 token chunks for projections

F32 = mybir.dt.float32
BF16 = mybir.dt.bfloat16
NWARM = 8                   # PE p-state warmup matmuls

CFG_BF16 = dict()
CFG_SAFE = CFG_FAST = CFG_F32R = CFG_BF16


def _emit(tc, xT, wqT, wkT, wvT, woT, y):
    nc = tc.nc
    scale = 1.0 / math.sqrt(D)

    with ExitStack() as ctx:
        consts = ctx.enter_context(tc.tile_pool(name="consts", bufs=1))
        wpool = ctx.enter_context(tc.tile_pool(name="wpool", bufs=1))
        xpool = ctx.enter_context(tc.tile_pool(name="xpool", bufs=2))
        arrs = ctx.enter_context(tc.tile_pool(name="arrs", bufs=1))
        ptpool = ctx.enter_context(tc.tile_pool(name="ptpool", bufs=2))
        smalls = ctx.enter_context(tc.tile_pool(name="smalls", bufs=3))
        ypool = ctx.enter_context(tc.tile_pool(name="ypool", bufs=6))
        psum = ctx.enter_context(tc.tile_pool(name="psum", bufs=1, space="PSUM"))

        # ---- constants (wtmp first: warmup matmuls gate on it) ----
        wtmp = consts.tile([P, IC], BF16, tag="wtmp", name="wtmp")
        nc.vector.memset(wtmp, 0.125)
        ones_col = consts.tile([P, 1], BF16, tag="ones", name="ones")
        nc.vector.memset(ones_col, 1.0)
        # tri_mask[p, i] = 1.0 if i >= p else 0 (keep lower triangle of S)
        tri_mask = consts.tile([P, P], BF16, tag="trimask", name="trimask")
        nc.gpsimd.memset(tri_mask, 0.0)
        nc.gpsimd.affine_select(
            out=tri_mask, in_=tri_mask, compare_op=mybir.AluOpType.is_gt,
            fill=1.0, base=0, pattern=[[-1, P]], channel_multiplier=1,
        )

        # ---- weight + x DMAs (host pre-tiled: contiguous per partition) ----
        w_sb = {}
        for nm in ("q", "k", "v"):
            w_sb[nm] = wpool.tile([P, KD, HPC * D], BF16, tag=f"w{nm}",
                                  name=f"w{nm}")
        xt_tiles = {}

        def issue_x_dma(c, parts=1):
            xt = xpool.tile([P, KD, IC], BF16, tag="xt", name=f"xt{c}")
            src = xT[c * P:(c + 1) * P].rearrange("p (ko t) -> p ko t", ko=KD)
            kq = KD // parts
            for pp in range(parts):
                nc.sync.dma_start(xt[:, pp * kq:(pp + 1) * kq],
                                  src[:, pp * kq:(pp + 1) * kq])
            xt_tiles[c] = xt

        # first chunk's inputs stream in interleaved parts so the first
        # projection matmuls start as soon as part 0 lands and chunk 0's
        # kt-interleaved consumption never outruns the DMA stream
        wq_src = wqT.rearrange("p (ko o) -> p ko o", ko=KD)
        xt0 = xpool.tile([P, KD, IC], BF16, tag="xt", name="xt0")
        x0_src = xT[0:P].rearrange("p (ko t) -> p ko t", ko=KD)
        nc.sync.dma_start(w_sb["q"][:, 0:4], wq_src[:, 0:4])
        nc.sync.dma_start(xt0[:, 0:4], x0_src[:, 0:4])
        nc.sync.dma_start(w_sb["q"][:, 4:8], wq_src[:, 4:8])
        nc.sync.dma_start(xt0[:, 4:8], x0_src[:, 4:8])
        nc.sync.dma_start(w_sb["q"][:, 8:], wq_src[:, 8:])
        nc.sync.dma_start(xt0[:, 8:12], x0_src[:, 8:12])
        nc.sync.dma_start(xt0[:, 12:], x0_src[:, 12:])
        xt_tiles[0] = xt0
        nc.sync.dma_start(w_sb["k"], wkT.rearrange("p (ko o) -> p ko o", ko=KD))
        nc.sync.dma_start(w_sb["v"], wvT.rearrange("p (ko o) -> p ko o", ko=KD))
        wo_sb = wpool.tile([P, HPC, D_MODEL], BF16, tag="wo", name="wo")
        nc.sync.dma_start(wo_sb, woT.rearrange("p (h m) -> p h m", h=HPC))

        # ---- PE warmup: ramp p-state while the first DMAs stream ----
        ps_w = psum.tile([P, IC], F32, tag="pa", name="warm")
        for _ in range(NWARM):
            nc.tensor.matmul(ps_w, wtmp[:, :P], wtmp, start=True, stop=True)

        # ---- persistent sbuf arrays ----
        qT = [arrs.tile([P, BT], BF16, tag=f"qT{h}", name=f"qT{h}")
              for h in range(HPC)]
        kT = [arrs.tile([P, BT], BF16, tag=f"kT{h}", name=f"kT{h}")
              for h in range(HPC)]
        v2_sb = arrs.tile([P, B, TJ, HPC, D], BF16, tag="v2", name="v2")
        outT = [arrs.tile([P, BT], BF16, tag=f"o{h}", name=f"o{h}")
                for h in range(HPC)]

        # ================= projection micro-unit FIFO =================
        # each unit is a closure emitting ~2 matmuls; markers record chunk
        # completion so attention can wait for its causal prerequisites.
        fifo = []
        done = set()

        def build_chunk(c, cnext):
            items = []
            b = c // 4
            jt0 = (c % 4) * 4
            tsl = slice(c * IC, (c + 1) * IC)

            if cnext is not None:
                items.append(lambda c=cnext: issue_x_dma(c))

            # q/k dests: [128 d, 512 tokens] accumulated over 16 k-tiles;
            # q dests first so chunk 0 matches weight DMA arrival order
            for di, (nm, h) in enumerate(
                    (("q", 0), ("q", 1), ("k", 0), ("k", 1))):
                tag = "pa" if di % 2 == 0 else "pb"
                st = {}
                for u in range(KD // 2):
                    def f(u=u, nm=nm, h=h, tag=tag, st=st, c=c):
                        if u == 0:
                            st["ps"] = psum.tile([P, IC], F32, tag=tag,
                                                 name=f"pj{c}")
                        ps = st["ps"]
                        for kt in (2 * u, 2 * u + 1):
                            nc.tensor.matmul(
                                ps, w_sb[nm][:, kt, h * D:(h + 1) * D],
                                xt_tiles[c][:, kt],
                                start=(kt == 0), stop=(kt == KD - 1),
                                skip_group_check=True)
                    items.append(f)

                def fcopy(nm=nm, h=h, st=st, tsl=tsl):
                    dst = qT[h] if nm == "q" else kT[h]
                    nc.vector.tensor_copy(dst[:, tsl], st["ps"])
                items.append(fcopy)

            # v dests: [128 tokens, 256 d] directly transposed (x stationary)
            for t4 in range(4):
                tag = "va" if t4 % 2 == 0 else "vb"
                st = {}
                for u in range(KD // 2):
                    def f(u=u, t4=t4, tag=tag, st=st, c=c):
                        if u == 0:
                            st["ps"] = psum.tile([P, IC], F32, tag=tag,
                                                 name=f"pv{c}")
                        ps = st["ps"]
                        for kt in (2 * u, 2 * u + 1):
                            nc.tensor.matmul(
                                ps[:, :HPC * D],
                                xt_tiles[c][:, kt, t4 * P:(t4 + 1) * P],
                                w_sb["v"][:, kt],
                                start=(kt == 0), stop=(kt == KD - 1),
                                skip_group_check=True)
                    items.append(f)

                def fcopy(t4=t4, st=st, b=b, jt0=jt0):
                    for h in range(HPC):
                        nc.vector.tensor_copy(
                            v2_sb[:, b, jt0 + t4, h],
                            st["ps"][:, h * D:(h + 1) * D])
                items.append(fcopy)

            items.append(lambda c=c: done.add(c))
            return items

        # ============== out-projection units + cast queue ==============
        unit_pool = []
        cast_q = []
        ustate = {"u": 0, "c": 0, "y_sb": None, "deep": False}

        def make_units(ic, b):
            t0 = (b * T + ic * IC) // P
            for tt in range(t0, t0 + IC // P):
                for mc in range(D_MODEL // IC):
                    unit_pool.append((tt, mc))

        def emit_cast():
            # gpsimd cannot read PSUM: casts alternate vector/scalar
            tt, mc, y_sb, ps_y, u = cast_q.pop(0)
            msl = slice(mc * IC, (mc + 1) * IC)
            c = ustate["c"]; ustate["c"] += 1
            # during attention scalar is exp-paced, so vector takes 2/3 of
            # the casts; in the exp-free final drain both engines are free
            # and a 50/50 split keeps vector off the critical path
            on_vec = (c % 2 == 0) if ustate["deep"] else (c % 3 != 2)
            if on_vec:
                nc.vector.tensor_copy(y_sb[:, msl], ps_y)
            else:
                nc.scalar.copy(y_sb[:, msl], ps_y)
            # dma_start costs ~0.6us of sync-queue time regardless of size;
            # two column-half DMAs per token tile spread all 16 engines
            if mc == D_MODEL // IC - 1:
                half = D_MODEL // 2
                nc.sync.dma_start(y[tt * P:(tt + 1) * P, :half],
                                  y_sb[:, :half])
                nc.sync.dma_start(y[tt * P:(tt + 1) * P, half:],
                                  y_sb[:, half:])

        def emit_unit(deep):
            # casts are deferred one slot so the cast queues never block on
            # ps_y matmuls (convoy avoidance)
            tt, mc = unit_pool.pop(0)
            u = ustate["u"]; ustate["u"] += 1
            msl = slice(mc * IC, (mc + 1) * IC)
            if mc == 0:
                ustate["y_sb"] = ypool.tile([P, D_MODEL], BF16, tag="ysb",
                                            name="ysb")
            y_sb = ustate["y_sb"]
            # units are only emitted once the proj FIFO is empty, so all
            # four proj banks are free for a deep rotation
            ytag = ["pa", "pb", "va", "vb"][u % 4]
            ps_y = psum.tile([P, IC], F32, tag=ytag, name="yps")
            for h in range(HPC):
                nc.tensor.matmul(
                    ps_y, outT[h][:, tt * P:(tt + 1) * P], wo_sb[:, h, msl],
                    start=(h == 0), stop=(h == HPC - 1))
            cast_q.append((tt, mc, y_sb, ps_y, u))
            if deep:
                emit_cast()

        def pull():
            # one PE filler micro-unit: proj FIFO first, then outproj units
            if fifo:
                fifo.pop(0)()
            elif unit_pool:
                emit_unit(deep=False)

        def drain_until(c):
            while c not in done and fifo:
                fifo.pop(0)()

        # ===================== attention chunk =====================
        def attn_chunk(ck, ic, h, b):
            i0 = b * T + ic * IC
            nj = 4 * (ic + 1)
            lo_of = lambda jt: max(jt - 4 * ic, 0) * P
            pt = ptpool.tile([P, 16 * IC], BF16, tag="pt", name="pt")

            def s_tile(jt):
                lo = lo_of(jt)
                ps_s = psum.tile([P, IC], F32, tag=f"s{jt % 2}",
                                 name=f"s{jt % 2}")
                nc.tensor.matmul(
                    ps_s[:, lo:],
                    kT[h][:, b * T + jt * P: b * T + (jt + 1) * P],
                    qT[h][:, i0 + lo: i0 + IC], start=True, stop=True)
                nc.scalar.activation(
                    pt[:, jt * IC + lo:(jt + 1) * IC], ps_s[:, lo:],
                    mybir.ActivationFunctionType.Exp, scale=scale)
                if jt - 4 * ic >= 0:
                    nc.vector.tensor_tensor(
                        pt[:, jt * IC + lo: jt * IC + lo + P],
                        pt[:, jt * IC + lo: jt * IC + lo + P],
                        tri_mask, mybir.AluOpType.mult)

            # denominator accumulates on DVE (bf16), freeing the PE
            pt_acc = smalls.tile([P, IC], BF16, tag="ptacc", name="ptacc")
            # PV double-buffered (o0/o1) so the norm chain of chunk n
            # overlaps chunk n+1's PV accumulation
            ps_o = psum.tile([P, IC], F32, tag=f"o{ck % 2}",
                             name=f"o{ck % 2}")

            # software pipeline: S runs one j-tile ahead of PV; a filler
            # micro-unit leads every slot so the PE queue stays dense
            s_tile(0)
            for jt in range(nj):
                if cast_q:
                    emit_cast()
                pull()
                if jt + 1 < nj:
                    s_tile(jt + 1)
                lo = lo_of(jt)
                psl = slice(jt * IC + lo, (jt + 1) * IC)
                if jt == 0:
                    nc.vector.tensor_copy(pt_acc, pt[:, psl])
                else:
                    nc.vector.tensor_tensor(
                        pt_acc[:, lo:], pt_acc[:, lo:], pt[:, psl],
                        mybir.AluOpType.add)
                nc.tensor.matmul(
                    ps_o[:, lo:], v2_sb[:, b, jt, h], pt[:, psl],
                    start=(jt == 0), stop=(jt == nj - 1),
                    skip_group_check=True)

            # denominator: ones-matmul into the cold s-bank row, then
            # broadcast + reciprocal + normalize off the critical path
            # (gpsimd partition_all_reduce was tried here and is too slow)
            ps_d = psum.tile([P, IC], F32, tag="s0", name="s0d")
            r = (ck % 3) * 32
            nc.tensor.matmul(ps_d[r:r + 1], ones_col, pt_acc,
                             start=True, stop=True, skip_group_check=True)
            den_sb = smalls.tile([1, IC], F32, tag="densb", name="densb")
            nc.vector.tensor_copy(den_sb, ps_d[r:r + 1])
            bc = smalls.tile([P, IC], F32, tag="bc", name="bc")
            nc.gpsimd.partition_broadcast(bc, den_sb)
            rb = smalls.tile([P, IC], F32, tag="rb", name="rb")
            nc.vector.reciprocal_approx_fast(out=rb, in_=bc)
            nc.vector.tensor_tensor(
                outT[h][:, i0:i0 + IC], ps_o, rb, mybir.AluOpType.mult)

        def emit_chunk0():
            # chunk 0 is DMA-bandwidth-bound: interleave kt across dest
            # pairs (q0/q1, then k0/k1, then v pairs) so each 0.5MB x part
            # is consumed over ~1.7us vs its ~1.4us transfer
            issue_x_dma(4)
            # all 8 psum banks are free during chunk 0, so each pair gets
            # fresh banks and no copy-vs-matmul WAR bubbles
            for pair, tags in (((("q", 0), ("q", 1)), ("pa", "pb")),
                               ((("k", 0), ("k", 1)), ("s0", "s1"))):
                ps = [psum.tile([P, IC], F32, tag=t, name="pj0") for t in tags]
                for kt in range(KD):
                    for (nm, h), p in zip(pair, ps):
                        nc.tensor.matmul(
                            p, w_sb[nm][:, kt, h * D:(h + 1) * D],
                            xt_tiles[0][:, kt],
                            start=(kt == 0), stop=(kt == KD - 1),
                            skip_group_check=True)
                for (nm, h), p in zip(pair, ps):
                    dst = qT[h] if nm == "q" else kT[h]
                    nc.vector.tensor_copy(dst[:, :IC], p)
            for t4s, vtags in (((0, 1), ("va", "vb")), ((2, 3), ("o0", "o1"))):
                ps = [psum.tile([P, IC], F32, tag=t, name="pv0")
                      for t in vtags]
                for kt in range(KD):
                    for t4, p in zip(t4s, ps):
                        nc.tensor.matmul(
                            p[:, :HPC * D],
                            xt_tiles[0][:, kt, t4 * P:(t4 + 1) * P],
                            w_sb["v"][:, kt],
                            start=(kt == 0), stop=(kt == KD - 1),
                            skip_group_check=True)
                for t4, p in zip(t4s, ps):
                    for h in range(HPC):
                        nc.vector.tensor_copy(v2_sb[:, 0, t4, h],
                                              p[:, h * D:(h + 1) * D])
            done.add(0)

        # ===================== schedule =====================
        # proj chunks stream batch-interleaved (0,4,1,5,...) so that proj
        # filler work lasts until the final attention group; attention
        # groups follow the same interleave, each gated on its causal
        # prerequisite chunk via drain_until.
        cord = (0, 4, 1, 5, 2, 6, 3, 7)
        emit_chunk0()
        for i in range(1, TCH):
            cnext = cord[i + 1] if i + 1 < TCH else None
            fifo.extend(build_chunk(cord[i], cnext))

        ck = 0
        for ic in range(NI):
            for b in range(B):
                drain_until(b * 4 + ic)
                for h in range(HPC):
                    attn_chunk(ck, ic, h, b)
                    ck += 1
                make_units(ic, b)
        while fifo:
            fifo.pop(0)()
        ustate["deep"] = True
        while cast_q:
            emit_cast()
        while unit_pool:
            emit_unit(deep=True)


def _build():
    nc = bacc.Bacc("TRN2", target_bir_lowering=False, debug=False,
                   num_devices=NCORES)
    # host pre-tiles everything so each DMA is contiguous per partition
    xT = nc.dram_tensor("xT", [TCH * P, KD * IC], BF16,
                        kind="ExternalInput").ap()
    wqT = nc.dram_tensor("wqT", [P, KD * HPC * D], BF16,
                         kind="ExternalInput").ap()
    wkT = nc.dram_tensor("wkT", [P, KD * HPC * D], BF16,
                         kind="ExternalInput").ap()
    wvT = nc.dram_tensor("wvT", [P, KD * HPC * D], BF16,
                         kind="ExternalInput").ap()
    woT = nc.dram_tensor("woT", [P, HPC * D_MODEL], BF16,
                         kind="ExternalInput").ap()
    y = nc.dram_tensor("y", [BT, D_MODEL], BF16, kind="ExternalOutput").ap()
    with tile.TileContext(nc) as tc:
        _emit(tc, xT, wqT, wkT, wvT, woT, y)
    nc.compile()
    return nc


def _prep_inputs(x, Wq, Wk, Wv, Wo):
    bf = ml_dtypes.bfloat16

    def wtile(w):  # [D_MODEL, HPC*D] -> [P, KD*(HPC*D)], contiguous rows
        return np.ascontiguousarray(
            w.reshape(KD, P, HPC * D).transpose(1, 0, 2).reshape(P, -1)
        ).astype(bf)

    xT2 = np.asarray(x, np.float32).reshape(BT, D_MODEL).T  # [D_MODEL, BT]
    xT = np.ascontiguousarray(
        xT2.reshape(KD, P, TCH, IC).transpose(2, 1, 0, 3).reshape(TCH * P, -1)
    ).astype(bf)
    in_maps = []
    for c in range(NCORES):
        rows = slice(c * HPC * D, (c + 1) * HPC * D)
        woT2 = np.asarray(Wo)[:, rows].T  # [HPC*D, D_MODEL]
        in_maps.append({
            "xT": xT,
            "wqT": wtile(np.asarray(Wq)[rows].T),
            "wkT": wtile(np.asarray(Wk)[rows].T),
            "wvT": wtile(np.asarray(Wv)[rows].T),
            "woT": np.ascontiguousarray(
                woT2.reshape(HPC, P, D_MODEL).transpose(1, 0, 2)
                .reshape(P, -1)).astype(bf),
        })
    return in_maps


def run(x, Wq, Wk, Wv, Wo, cfg=None, trace=False):
    nc = _build()
    in_maps = _prep_inputs(x, Wq, Wk, Wv, Wo)
    try:
        res = run_bass_kernel_spmd(nc, in_maps, core_ids=list(range(NCORES)),
                                   trace=trace)
    except Exception:
        res = run_bass_kernel_spmd(nc, in_maps, core_ids=list(range(NCORES)),
                                   trace=trace)
    y = np.zeros((BT, D_MODEL), np.float32)
    for r in res.results:
        y += np.asarray(r["y"], dtype=np.float32)
    return y.reshape(B, T, D_MODEL), res


def kernel(x, Wq, Wk, Wv, Wo):
    y, _ = run(x, Wq, Wk, Wv, Wo)
    return y
